# revision 4
# baseline (speedup 1.0000x reference)
"""Block-diagonal attention kernel for Trainium2 (8 NeuronCores).

Problem: q,k,v [4, 16, 4128, 64] f32. Attention within consecutive 64-row
blocks (64 full blocks + one 32-row remainder per (b,h); 4128 = 64*64+32).

Design (v3):
- Host casts to bf16 and pre-lays-out everything; device DMAs are all
  contiguous per partition. Ones column for row sums is baked into V on
  the host (65-wide V rows), so PV matmuls emit outputs AND row sums.
- Q,K pre-transposed on host to [d, n]; head pair stacked on partition
  halves -> QK matmuls for the two heads run on different row groups
  (concurrent), S^T for block 2c lands on partitions 0:64 via matmul col
  group 0 and block 2c+1 on 64:128 via col group 64.
- One exp activation instr per head-superchunk over [128, 8*64] of pure
  payload (no masking, no garbage).
- PV per block: 64-contraction matmul on one partition half (row groups
  alternate -> concurrent); t=0/t=1 write different PSUM bank groups.
- Normalize: one reciprocal + two broadcast tensor_tensor muls per
  pair-superchunk, all full 128-partition width.

Sharding: 64 (b,h) pairs -> 8 per core = 4 head-pairs.
Softmax max-subtraction skipped: scores ~ N(0,1), exp is safe in f32.
"""
import sys

sys.path.insert(0, "/opt/trn_rl_repo")

import numpy as np
from contextlib import ExitStack

import concourse.tile as tile
from concourse import bacc, mybir
from concourse.bass import broadcast_tensor_aps
from concourse.bass_utils import run_bass_kernel_spmd

F32 = mybir.dt.float32
BF16 = mybir.dt.bfloat16
AF = mybir.ActivationFunctionType
MUL = mybir.AluOpType.mult
NPBF = mybir.dt.np(mybir.dt.bfloat16)

B, H, N, D = 4, 16, 4128, 64
NMAIN = 4096
PAIRS = 4                # head pairs per core (8 heads)
N_SC = 4                 # superchunks per head; 1024 rows = 8 block-pairs
BP = 8                   # 128-row block-pairs per superchunk
SCALE = 1.0 / 8.0

BIG_BUFS = 4
SS_BUFS = 2


def _qk_exp(nc, sb, ps, qtb, ktb, s):
    """QK + exp for one superchunk (1024 rows) of both heads -> pt tiles."""
    pts = []
    for j in range(2):
        p0 = 64 * j
        ss = ps.tile([128, BP, 64], F32, tag=f"ss{j}", bufs=SS_BUFS)
        for bp in range(BP):
            for t in range(2):
                c0 = 1024 * s + 128 * bp + 64 * t
                nc.tensor.matmul(ss[64 * t:64 * t + 64, bp, :],
                                 ktb[p0:p0 + 64, c0:c0 + 64],
                                 qtb[p0:p0 + 64, c0:c0 + 64],
                                 tile_position=(p0, 64 * t))
        pt = sb.tile([128, BP, 64], BF16, tag=f"pt{j}")
        nc.scalar.activation(pt[:], ss[:], AF.Exp, scale=SCALE)
        pts.append(pt)
    return pts


def _pv_norm(nc, sb, ps, pts, vb, ob, s):
    """PV + normalize for superchunk s (issued one stage behind QK/exp so
    the PE never waits on the exp latency).

    PV: per block one 64-contraction matmul; rhs includes the baked ones
    column so col 64 is the row sum. t=0 -> PSUM bank 0, t=1 -> bank 1.
    Half-superchunk granularity (4 block-pairs) so o fits 2 PSUM banks
    with bufs=2 -> PV(next) overlaps norm(current) instead of ping-pong."""
    for h in range(2):
        o = ps.tile([128, 2, 4, 128], F32, tag="o", bufs=2)
        for bp4 in range(4):
            bp = 4 * h + bp4
            for j in range(2):
                for t in range(2):
                    nc.tensor.matmul(o[64 * j:64 * j + 64, t, bp4, 0:65],
                                     pts[j][64 * t:64 * t + 64, bp, :],
                                     vb[64 * t:64 * t + 64, 8 * s + bp, j, :],
                                     tile_position=(64 * t, 64 * j))

        r = sb.tile([128, 2, 4, 1], F32, tag="r")
        nc.vector.reciprocal(r[:, :, :, 0], o[:, :, :, 64])
        b0 = 16 * s + 8 * h
        obv = ob[:, b0:b0 + 8, :].rearrange("p (b t) d -> p t b d", t=2)
        o_ap, r_ap = broadcast_tensor_aps(o[:, :, :, 0:64], r[:])
        nc.vector.tensor_tensor(obv[:], o_ap, r_ap, op=MUL)


def _remainder(nc, sb, ps, qtb, ktb, rvb, routs, p):
    """Both heads' 32-row remainder blocks for pair p. QK of the two heads
    is on different row groups (concurrent) -> different banks ss0/ss1."""
    ptr = sb.tile([32, 2, 32], BF16, tag="ptr")
    for j in range(2):
        ss = ps.tile([128, BP, 64], F32, tag=f"ss{j}", bufs=SS_BUFS)
        nc.tensor.matmul(ss[0:32, 0, 0:32],
                         ktb[64 * j:64 * j + 64, NMAIN:N],
                         qtb[64 * j:64 * j + 64, NMAIN:N],
                         tile_position=(64 * j, 0))
        nc.scalar.activation(ptr[:, j, :], ss[0:32, 0, 0:32], AF.Exp,
                             scale=SCALE)

    o = ps.tile([128, 2, 4, 128], F32, tag="o", bufs=2)
    for j in range(2):
        nc.tensor.matmul(o[0:32, 0, j, 0:65], ptr[:, j, :], rvb[:, j, :],
                         tile_position=(0, 0))
    rr = sb.tile([32, 2], F32, tag="rr")
    nc.vector.reciprocal(rr[:], o[0:32, 0, 0:2, 64])
    for j in range(2):
        nc.vector.tensor_scalar_mul(routs[:, 2 * p + j, :],
                                    o[0:32, 0, j, 0:64], rr[:, j:j + 1])


def build_nc(repeat=1):
    nc = bacc.Bacc("TRN2", target_bir_lowering=False, debug=False, num_devices=8)
    qt = nc.dram_tensor("qt", [PAIRS, 128, N], BF16, kind="ExternalInput").ap()
    kt = nc.dram_tensor("kt", [PAIRS, 128, N], BF16, kind="ExternalInput").ap()
    vc = nc.dram_tensor("vc", [PAIRS, 128, 32, 2, 65], BF16,
                        kind="ExternalInput").ap()
    vr = nc.dram_tensor("vr", [PAIRS, 32, 2, 65], BF16,
                        kind="ExternalInput").ap()
    oc = nc.dram_tensor("oc", [PAIRS, 128, NMAIN], BF16,
                        kind="ExternalOutput").ap()
    orm = nc.dram_tensor("orm", [32, 8, 64], BF16, kind="ExternalOutput").ap()

    with tile.TileContext(nc) as tc, ExitStack() as ctx:
        singles = ctx.enter_context(tc.tile_pool(name="singles", bufs=1))
        big = ctx.enter_context(tc.tile_pool(name="big", bufs=BIG_BUFS))
        sb = ctx.enter_context(tc.tile_pool(name="sb", bufs=3))
        ps = ctx.enter_context(tc.tile_pool(name="ps", bufs=2, space="PSUM"))

        routs = singles.tile([32, 8, 64], BF16)

        # Warm the ACT exp table at t=0 (overlaps the first input DMAs);
        # otherwise the first real exp pays the ~2.7us ACT_TABLE_LOAD.
        warm = singles.tile([32, 1], F32)
        nc.vector.memset(warm[:], 0.0)
        nc.scalar.activation(warm[:], warm[:], AF.Exp)

        def load_pair(p, first=False):
            qtb = big.tile([128, N], BF16, tag="qtb")
            ktb = big.tile([128, N], BF16, tag="ktb")
            vb = big.tile([128, 32, 2, 65], BF16, tag="vb")
            rvb = sb.tile([32, 2, 65], BF16, tag="rvb")
            # Spread loads over three independent DMA rings (SP-HWDGE,
            # ACT-HWDGE, Pool-SWDGE): two rings alone must each sustain
            # exactly half the 358 GB/s HBM cap, so any interleave hiccup
            # costs bandwidth; three rings leave slack. On HW the ACT
            # HWDGE ring drains independently of ACT compute (exp).
            nc.sync.dma_start(out=qtb[:], in_=qt[p])
            nc.scalar.dma_start(out=ktb[:], in_=kt[p])
            # first pair: v behind qt on sync so q/k land first (the first
            # PV is deferred 2 superchunks, so v still lands in time)
            veng = nc.sync if first else nc.gpsimd
            veng.dma_start(out=vb[:], in_=vc[p])
            nc.scalar.dma_start(out=rvb[:], in_=vr[p])
            return qtb, ktb, vb, rvb

        # Software pipeline: issue pair p+1's loads before pair p's output
        # DMA so the pool queue's FIFO never blocks a prefetch behind the
        # output's wait-for-ob dependency.
        total = repeat * PAIRS
        loaded = [load_pair(0, first=True)]
        pend = []
        nout = [0]

        def flush_one(last=False):
            pts, vb2, ob2, s2, p2 = pend.pop(0)
            _pv_norm(nc, sb, ps, pts, vb2, ob2, s2)
            if s2 == N_SC - 1:
                # ob2 is complete once this PV+norm lands
                dst = oc[p2].rearrange("q (b d) -> q b d", d=64)
                if last:
                    # split the final (tail) store across two queues
                    nc.gpsimd.dma_start(out=dst[:, 0:32], in_=ob2[:, 0:32])
                    nc.sync.dma_start(out=dst[:, 32:64], in_=ob2[:, 32:64])
                else:
                    # rotate stores across all three rings (~1.4 MB/pair
                    # per ring including loads)
                    oeng = (nc.gpsimd, nc.sync, nc.scalar)[nout[0] % 3]
                    nout[0] += 1
                    oeng.dma_start(out=dst, in_=ob2[:])

        # PV runs two superchunks behind QK/exp so the PE never waits on
        # the serialized exp latency (2x ~720ns per superchunk on ACT).
        for it in range(total):
            p = it % PAIRS
            qtb, ktb, vb, rvb = loaded.pop(0)
            ob = big.tile([128, 64, 64], BF16, tag="ob")
            for s in range(N_SC):
                pts = _qk_exp(nc, sb, ps, qtb, ktb, s)
                pend.append((pts, vb, ob, s, p))
                if len(pend) > 2:
                    flush_one()
            if it + 1 < total:
                loaded.append(load_pair((it + 1) % PAIRS))
            _remainder(nc, sb, ps, qtb, ktb, rvb, routs, p)
        while pend:
            flush_one(last=(len(pend) == 1))
        nc.scalar.dma_start(out=orm[:], in_=routs[:])

    nc.compile()
    return nc


_CACHE = {}


def _prep_core(q8, k8, v8):
    """Host-side layout prep for one core's 8 heads ([8, 4128, 64] f32)."""
    qt = np.ascontiguousarray(
        q8.transpose(0, 2, 1).reshape(PAIRS, 128, N).astype(NPBF))
    kt = np.ascontiguousarray(
        k8.transpose(0, 2, 1).reshape(PAIRS, 128, N).astype(NPBF))
    vbf = v8.astype(NPBF)
    vcm = np.ones((PAIRS, 128, 32, 2, 65), dtype=NPBF)
    vcm[..., 0:64] = (vbf[:, :NMAIN, :].reshape(PAIRS, 2, 32, 128, 64)
                      .transpose(0, 3, 2, 1, 4))
    vrm = np.ones((PAIRS, 32, 2, 65), dtype=NPBF)
    vrm[..., 0:64] = (vbf[:, NMAIN:, :].reshape(PAIRS, 2, 32, 64)
                      .transpose(0, 2, 1, 3))
    return {"qt": qt, "kt": kt, "vc": vcm, "vr": vrm}


def kernel(q, k, v):
    assert q.shape == (B, H, N, D), q.shape
    if "nc" not in _CACHE:
        _CACHE["nc"] = build_nc()
    nc = _CACHE["nc"]

    q64 = np.asarray(q, dtype=np.float32).reshape(B * H, N, D)
    k64 = np.asarray(k, dtype=np.float32).reshape(B * H, N, D)
    v64 = np.asarray(v, dtype=np.float32).reshape(B * H, N, D)

    in_maps = [_prep_core(q64[8 * i:8 * i + 8], k64[8 * i:8 * i + 8],
                          v64[8 * i:8 * i + 8]) for i in range(8)]

    try:
        res = run_bass_kernel_spmd(nc, in_maps, core_ids=list(range(8)))
    except Exception:
        import time
        time.sleep(2.0)
        res = run_bass_kernel_spmd(nc, in_maps, core_ids=list(range(8)))

    out = np.empty((B * H, N, D), dtype=np.float32)
    for i in range(8):
        oc = np.asarray(res.results[i]["oc"], dtype=np.float32)
        orm = np.asarray(res.results[i]["orm"], dtype=np.float32)
        # oc [4, 128(64j+r), 4096(64b+d)] -> [8 heads, 4096 rows, 64]
        oh = (oc.reshape(PAIRS, 2, 64, 64, 64).transpose(0, 1, 3, 2, 4)
              .reshape(8, NMAIN, D))
        out[8 * i:8 * i + 8, :NMAIN, :] = oh
        out[8 * i:8 * i + 8, NMAIN:, :] = orm.transpose(1, 0, 2)
    return out.reshape(B, H, N, D)
